# revision 15
# baseline (speedup 1.0000x reference)
"""GAdapter (GNN message passing + adapter MLP) Bass kernel for Trainium2, 8-core SPMD.

Entry point: kernel(**inputs) -> np.ndarray [1, N, H] float32.

Two-launch design (no collectives), aggregation in the down-projected space.

k1 (per core): LN of own x slab (f16 input) -> eta (residual, f16) and
    d = eta @ (diag(pre_g) down_w^T) [+ pre_b down_w^T]   [SLAB, 32] f16.
    Stats are computed 4 tiles at a time (bn_stats groups), eta is transposed
    with the DMA xbar (dma_start_transpose on SP/Act queues, no PE transpose /
    PSUM eviction), and the four down-proj matmul results of a quad share one
    PSUM bank so a single activation Copy evicts them.
Host: concat d slabs -> table [N+1, 128] f16 (first 32 cols = payload, last
    row zero sentinel), viewed as int32 [N+1, 64] (256B rows); replicated.
k2 (per core): per 128-row tile, gather 64B per edge slot (elem_size=16 int32
    on a 256B row stride). Claim packing gives every row exactly C0=8 identity
    claim groups (G=4 edge slots each, claim k == row k), so the routing
    matmul per group is matmul(psAT, lhsT=payload_g, rhs=identity) -- the
    payload transpose-accumulates into psAT[je*32+c, m] with NO one-hot
    build. Rows with >C0*G edges spill into 1-2 overflow groups routed by a
    one-hot rhs (iota==R on DVE), the only per-tile DVE routing work.
    psAT feeds the up-projection directly as lhsT (agg is already
    transposed), so no PE transpose exists in k2 at all. Tiles are processed
    in pairs: the pair's psAT/psZ live in one PSUM bank each, so one
    activation op per pair does the PSUM->SBUF copy / relu, and bn_stats
    runs on [128, 2, H] groups.

Claim layout: tile t has NG[t] = C0 + ngs_ov[t] groups of 128 claims x G=4
edge slots; slot s = (g*G+je)*128 + k holds edge je of claim (g, k); identity
groups have k == source row, overflow groups carry R[k, g] = source row.
"""

from contextlib import ExitStack
from dataclasses import dataclass, field

import numpy as np

import concourse.bass as bass
import concourse.tile as tile
from concourse import bacc, mybir

F32 = mybir.dt.float32
F16 = mybir.dt.float16
I32 = mybir.dt.int32
I64 = mybir.dt.int64
I16 = mybir.dt.int16
EPS = 1e-5


def _raw_dma_gather(g, out_ap, in_ap, idxs_ap, num_idxs, num_idxs_reg, elem_size,
                    elem_step, single_packet=False, queue_num=0):
    """dma_gather for sub-256B elements (elem read < 256B row stride).

    Same lowering as bass's dma_gather non-transpose DRAM path, minus the
    elem_size%256 assert (which only the transpose mode needs). The row
    stride (elem_step * dtype) must still be a 256B multiple.
    """
    from concourse.bass import exact_div

    stride_bytes = elem_step * mybir.dt.size(in_ap.dtype)
    stride_bytes_256 = exact_div(stride_bytes, 256)
    _in_ap = g.lower_ap_dma(in_ap, for_custom_bir_dma=True)
    _idxs_ap = g.lower_ap(idxs_ap)
    _out_ap = g.lower_ap(out_ap)
    return g.add_instruction(
        mybir.InstDMAGatherAnt(
            name=g.bass.get_next_instruction_name(),
            ins=[*_in_ap, _idxs_ap, g.lower_val_access(g.to_reg(num_idxs_reg))],
            outs=[_out_ap], transpose=False, num_idxs=num_idxs,
            elem_size=elem_size, stride_bytes_256=stride_bytes_256, gen_mode=0,
            single_packet=single_packet, queue_num=queue_num,
            sbuf_tokens_per_rank=0, sbuf_free_dim_per_rank=0,
            sbuf_free_dim_pad_per_rank=0, sbuf_byte_offset=0,
        ))


@dataclass
class Cfg:
    N: int = 16384
    H: int = 128
    B: int = 32
    NCORES: int = 8
    G: int = 4
    C0: int = 8                # identity claim groups per tile (C0*G edge cap/row)
    ngs_ov: tuple = field(default_factory=lambda: tuple([1] * 16))  # overflow groups/tile
    use_bM: bool = False       # pre_b != 0 (bias into d)
    use_pre_gb: bool = False   # pre_g/pre_b non-identity (residual adjust)
    use_c: bool = False        # down_b/up_b != 0
    use_post_gb: bool = False  # post_g/post_b non-identity
    reps: int = 1

    @property
    def SLAB(self):
        return self.N // self.NCORES

    @property
    def T(self):
        return self.SLAB // 128


def build_k1(cfg: Cfg):
    nc = bacc.Bacc("TRN2", target_bir_lowering=False, debug=False, num_devices=cfg.NCORES)
    H, B, T = cfg.H, cfg.B, cfg.T
    QN = T // 4  # quads
    x_in = nc.dram_tensor("x_slab", [cfg.SLAB, H], F16, kind="ExternalInput")
    ident_in = nc.dram_tensor("ident", [128, 128], F16, kind="ExternalInput")
    dwT_in = nc.dram_tensor("dwT", [H, B], F16, kind="ExternalInput")
    c1_in = nc.dram_tensor("c1", [1, B], F16, kind="ExternalInput")
    pre_g_in = nc.dram_tensor("pre_g", [1, H], F16, kind="ExternalInput")
    pre_b_in = nc.dram_tensor("pre_b", [1, H], F16, kind="ExternalInput")
    d_out = nc.dram_tensor("d", [128, T * B], F16, kind="ExternalOutput")
    eta_out = nc.dram_tensor("eta", [128, T * H], F16, kind="ExternalOutput")

    with tile.TileContext(nc) as tc, ExitStack() as ctx:
        const = ctx.enter_context(tc.tile_pool(name="const", bufs=1))
        statp = ctx.enter_context(tc.tile_pool(name="stat", bufs=3))
        etaTp = ctx.enter_context(tc.tile_pool(name="etaT", bufs=16))
        psD_p = ctx.enter_context(tc.tile_pool(name="psD", bufs=2, space="PSUM"))
        psT_p = ctx.enter_context(tc.tile_pool(name="psT", bufs=2, space="PSUM"))

        eps_t = const.tile([128, 1], F32)
        nc.vector.memset(eps_t[:], EPS)
        # the first Activation-engine instruction must be a Sqrt activation:
        # the act-table pass then loads only the sqrt set (which also serves
        # Copy/Relu) instead of a default set plus the sqrt set.
        warm_t = const.tile([128, 1], F32)
        nc.scalar.activation(warm_t[:], eps_t[:], mybir.ActivationFunctionType.Sqrt)
        dwT_t = const.tile([H, B], F16)
        ident_t = const.tile([128, 128], F16)
        if cfg.use_bM:
            ones1 = const.tile([1, 128], F16)
            nc.vector.memset(ones1[:], 1.0)
            c1_t = const.tile([1, B], F16)
            nc.scalar.dma_start(c1_t[:], c1_in[:])
        if cfg.use_pre_gb:
            # broadcast pre_g/pre_b to [128, H] via a K=1 matmul (done once)
            pre_g_t = const.tile([1, H], F16)
            nc.scalar.dma_start(pre_g_t[:], pre_g_in[:])
            pre_b_t = const.tile([1, H], F16)
            nc.scalar.dma_start(pre_b_t[:], pre_b_in[:])
            ones_c = const.tile([1, 128], F16)
            nc.vector.memset(ones_c[:], 1.0)
            ps_g = psD_p.tile([128, H], F32, tag="pro", padded_shape=[128, 512])
            nc.tensor.matmul(ps_g[:], ones_c[:], pre_g_t[:], start=True, stop=True)
            gb_t = const.tile([128, H], F16)
            nc.scalar.activation(gb_t[:], ps_g[:], mybir.ActivationFunctionType.Copy)
            ps_b = psD_p.tile([128, H], F32, tag="pro", padded_shape=[128, 512])
            nc.tensor.matmul(ps_b[:], ones_c[:], pre_b_t[:], start=True, stop=True)
            bb_t = const.tile([128, H], F16)
            nc.scalar.activation(bb_t[:], ps_b[:], mybir.ActivationFunctionType.Copy)

        for _rep in range(cfg.reps):
            x_all = const.tile([128, T, H], F16, name="x_all")
            x_src = x_in.ap().rearrange("(t p) h -> p t h", p=128)
            nc.sync.dma_start(x_all[:, 0:1, :], x_src[:, 0:1, :])
            nc.sync.dma_start(x_all[:, 1:4, :], x_src[:, 1:4, :])
            nc.sync.dma_start(x_all[:, 4:8, :], x_src[:, 4:8, :])
            nc.gpsimd.dma_start(x_all[:, 8:12, :], x_src[:, 8:12, :])
            nc.gpsimd.dma_start(x_all[:, 12:16, :], x_src[:, 12:16, :])
            nc.sync.dma_start(dwT_t[:], dwT_in[:])
            nc.gpsimd.dma_start(ident_t[:], ident_in[:])
            eta_all = const.tile([128, T, H], F16, name="eta_all")
            d_all = const.tile([128, T, B], F16, name="d_all")
            eta_dst = eta_out.ap().rearrange("p (t h) -> p t h", t=T)
            live = {}

            def stage_statsA(q):
                # hw verifier requires single-group bn_stats ([128, F] -> [128, 6])
                mvq = statp.tile([128, 4, 2], F32, tag="mv", name="mvq")
                for i in range(4):
                    st6 = statp.tile([128, 6], F32, tag="st", name="st6")
                    nc.vector.bn_stats(st6[:], x_all[:, 4 * q + i, :])
                    nc.vector.bn_aggr(mvq[:, i, :], st6[:])
                sd4 = statp.tile([128, 4, 1], F32, tag="sd", name="sd4")
                nc.scalar.activation(sd4[:], mvq[:, :, 1:2],
                                     mybir.ActivationFunctionType.Sqrt, bias=eps_t[:])
                live[q] = (mvq, sd4)

            def stage_statsB(q):
                mvq, sd4 = live[q]
                rs4 = statp.tile([128, 4, 1], F32, tag="rs", name="rs4")
                nc.vector.reciprocal(rs4[:], sd4[:])
                live[q] = (mvq, rs4)

            def stage_tile(t):
                q, i = t // 4, t % 4
                mvq, rs4 = live[q]
                eta = eta_all[:, t, :]
                nc.gpsimd.tensor_scalar(
                    eta, x_all[:, t, :], mvq[:, i, 0:1], rs4[:, i, 0:1],
                    mybir.AluOpType.subtract, mybir.AluOpType.mult,
                )
                etaT = etaTp.tile([128, H], F16, tag="etaT", name="etaT")
                if q == QN - 1:
                    # last quad: PE transpose + Act eviction -- shorter latency
                    # than the DMA xbar, so the kernel tail isn't gated on the
                    # ~1.7us DMA completion delay.
                    psT = psT_p.tile([128, H], F16, tag="psT",
                                     padded_shape=[128, 1024], name="psT")
                    nc.tensor.transpose(psT[:], eta, ident_t[:])
                    nc.scalar.activation(etaT[:], psT[:],
                                         mybir.ActivationFunctionType.Copy)
                else:
                    nc.sync.dma_start_transpose(etaT[:], eta)
                live[("etaT", t)] = etaT
                if cfg.use_pre_gb:
                    # residual eta gets the pre-LN affine; d path uses raw LN
                    # (pre_g/pre_b are folded into dwT/c1 on the host).
                    nc.vector.tensor_tensor(eta, eta, gb_t[:], mybir.AluOpType.mult)
                    nc.vector.tensor_tensor(eta, eta, bb_t[:], mybir.AluOpType.add)

            def stage_mm(t):
                q, i = t // 4, t % 4
                etaT = live.pop(("etaT", t))
                if i == 0:
                    live[("psD", q)] = psD_p.tile([128, 4, B], F32, tag="psD",
                                                  padded_shape=[128, 4, 128],
                                                  name="psDq")
                psDq = live[("psD", q)]
                nc.tensor.matmul(psDq[:, i, :], etaT[:], dwT_t[:], start=True,
                                 stop=not cfg.use_bM)
                if cfg.use_bM:
                    nc.tensor.matmul(psDq[:, i, :], ones1[:], c1_t[:], start=False,
                                     stop=True)

            def stage_dcopy(q):
                psDq = live.pop(("psD", q))
                nc.scalar.activation(d_all[:, 4 * q:4 * q + 4, :], psDq[:],
                                     mybir.ActivationFunctionType.Copy)

            stage_statsA(0)
            stage_statsA(1)
            for q in range(QN):
                stage_statsB(q)
                for i in range(4):
                    stage_tile(4 * q + i)
                if q + 2 < QN:
                    stage_statsA(q + 2)
                if q >= 1:
                    for i in range(4):
                        stage_mm(4 * (q - 1) + i)
                    stage_dcopy(q - 1)
                    del live[q - 1]
            for i in range(4):
                stage_mm(4 * (QN - 1) + i)
            # outputs last so they never head-block the transpose/copy queues
            nc.sync.dma_start(eta_dst[:, 0:4, :], eta_all[:, 0:4, :])
            nc.sync.dma_start(eta_dst[:, 4:8, :], eta_all[:, 4:8, :])
            nc.scalar.dma_start(eta_dst[:, 8:12, :], eta_all[:, 8:12, :])
            stage_dcopy(QN - 1)
            del live[QN - 1]
            nc.scalar.dma_start(eta_dst[:, 12:16, :], eta_all[:, 12:16, :])
            nc.gpsimd.dma_start(d_out.ap().rearrange("p (t b) -> p t b", t=T), d_all[:])

    nc.compile()
    return nc


def build_k2(cfg: Cfg):
    nc = bacc.Bacc("TRN2", target_bir_lowering=False, debug=False, num_devices=cfg.NCORES)
    H, B, T, G, C0 = cfg.H, cfg.B, cfg.T, cfg.G, cfg.C0
    ngs_ov = cfg.ngs_ov
    NG = [C0 + o for o in ngs_ov]
    NGmax = max(NG)
    tot_ov = sum(ngs_ov)
    tot_cols = sum(NG) * G * 8  # i16 idx cols: num_idxs/16 per tile

    table_in = nc.dram_tensor("table", [cfg.N + 1, 64], I32, kind="ExternalInput")
    eta_in = nc.dram_tensor("eta", [128, T * H], F16, kind="ExternalInput")
    idx_in = nc.dram_tensor("idx_plane", [128, tot_cols], I16, kind="ExternalInput")
    R_in = nc.dram_tensor("R_plane", [128, max(tot_ov, 1)], F32, kind="ExternalInput")
    iota_in = nc.dram_tensor("iota", [128, 128], F16, kind="ExternalInput")
    ident_in = nc.dram_tensor("ident", [128, 128], F16, kind="ExternalInput")
    upwT_in = nc.dram_tensor("upwT4", [128, H], F16, kind="ExternalInput")
    c_in = nc.dram_tensor("c_t", [1, H], F16, kind="ExternalInput")
    post_g_in = nc.dram_tensor("post_g", [1, H], F16, kind="ExternalInput")
    post_b_in = nc.dram_tensor("post_b", [1, H], F16, kind="ExternalInput")
    y_out = nc.dram_tensor("y", [128, T * H], F16, kind="ExternalOutput")

    icolb = [sum(NG[:i]) * G * 8 for i in range(T)]
    ovcolb = [sum(ngs_ov[:i]) for i in range(T)]

    with tile.TileContext(nc) as tc, ExitStack() as ctx:
        const = ctx.enter_context(tc.tile_pool(name="const", bufs=1))
        statp = ctx.enter_context(tc.tile_pool(name="stat", bufs=3))
        ohp = ctx.enter_context(tc.tile_pool(name="oh", bufs=8))
        gathp = ctx.enter_context(tc.tile_pool(name="gath", bufs=6))
        outp = ctx.enter_context(tc.tile_pool(name="outp", bufs=3))
        psA_p = ctx.enter_context(tc.tile_pool(name="psA", bufs=3, space="PSUM"))
        psZ_p = ctx.enter_context(tc.tile_pool(name="psZ", bufs=3, space="PSUM"))

        # const loads: Pool takes iota/R (before gathers), Act takes ident/upwT4,
        # SP streams the idx plane in 2-tile chunks, then eta halves.
        iota_t = const.tile([128, 128], F16)
        nc.gpsimd.dma_start(iota_t[:], iota_in[:])
        Rp_t = const.tile([128, max(tot_ov, 1)], F32)
        nc.gpsimd.dma_start(Rp_t[:], R_in[:])
        ident_t = const.tile([128, 128], F16)
        nc.scalar.dma_start(ident_t[:], ident_in[:])
        eps_t = const.tile([128, 1], F32)
        nc.vector.memset(eps_t[:], EPS)
        # first Act-engine instruction = Sqrt activation -> single table load
        # (no other Act-engine instruction may precede it, incl. DMAs)
        warm_t = const.tile([128, 1], F32)
        nc.scalar.activation(warm_t[:], eps_t[:], mybir.ActivationFunctionType.Sqrt)
        upwT_t = const.tile([128, H], F16)
        if cfg.use_c:
            ones1 = const.tile([1, 128], F16)
            nc.vector.memset(ones1[:], 1.0)
            c_t = const.tile([1, H], F16)
            nc.scalar.dma_start(c_t[:], c_in[:])
        if cfg.use_post_gb:
            ones_c = const.tile([1, 128], F16)
            nc.vector.memset(ones_c[:], 1.0)
            post_g_t = const.tile([1, H], F16)
            nc.scalar.dma_start(post_g_t[:], post_g_in[:])
            post_b_t = const.tile([1, H], F16)
            nc.scalar.dma_start(post_b_t[:], post_b_in[:])
            ps_g = psZ_p.tile([128, H], F32, tag="psZ", padded_shape=[128, 512])
            nc.tensor.matmul(ps_g[:], ones_c[:], post_g_t[:], start=True, stop=True)
            postg_b = const.tile([128, H], F16)
            nc.scalar.activation(postg_b[:], ps_g[:], mybir.ActivationFunctionType.Copy)
            ps_b = psZ_p.tile([128, H], F32, tag="psZ", padded_shape=[128, 512])
            nc.tensor.matmul(ps_b[:], ones_c[:], post_b_t[:], start=True, stop=True)
            postb_b = const.tile([128, H], F16)
            nc.scalar.activation(postb_b[:], ps_b[:], mybir.ActivationFunctionType.Copy)

        for _rep in range(cfg.reps):
            idxp_t = const.tile([128, tot_cols], I16, name="idxp")
            eta_all = const.tile([128, T, H], F16, name="eta_all")
            e_src = eta_in.ap().rearrange("p (t h) -> p t h", t=T)
            for c in range(0, T, 4):
                hi = icolb[c + 4] if c + 4 < T else tot_cols
                nc.sync.dma_start(idxp_t[:, icolb[c]:hi], idx_in[:, icolb[c]:hi])
                if c == 0:
                    nc.sync.dma_start(upwT_t[:], upwT_in[:])
                if c == 4:
                    nc.sync.dma_start(eta_all[:, 0:T // 2, :], e_src[:, 0:T // 2, :])
                if c == 8:
                    nc.sync.dma_start(eta_all[:, T // 2:T, :], e_src[:, T // 2:T, :])
            y_all = const.tile([128, T, H], F16, name="y_all")
            y_dst = y_out.ap().rearrange("p (t h) -> p t h", t=T)
            live = {}

            def stage_gather(t):
                n_idx = NG[t] * G * 128
                gath = gathp.tile([128, NGmax * G, 16], I32, tag="gath", name="gath")
                _raw_dma_gather(
                    nc.gpsimd,
                    gath[:, 0:NG[t] * G, :],
                    table_in.ap()[:, 0:16],
                    idxp_t[:, icolb[t]:icolb[t] + n_idx // 16],
                    num_idxs=n_idx,
                    num_idxs_reg=n_idx,
                    elem_size=16,
                    elem_step=64,
                )
                ohs = []
                for j in range(ngs_ov[t]):
                    oh = ohp.tile([128, 128], F16, tag="oh", name="oh")
                    nc.vector.tensor_scalar(
                        oh[:], iota_t[:], Rp_t[:, ovcolb[t] + j:ovcolb[t] + j + 1],
                        None, mybir.AluOpType.is_equal,
                    )
                    ohs.append(oh)
                live[t] = {"gath": gath, "ohs": ohs}

            def stage_route(P):
                psATp = psA_p.tile([128, 2, 128], F32, tag="psA",
                                   padded_shape=[128, 2, 256], name="psATp")
                for i in range(2):
                    t = 2 * P + i
                    st = live.pop(t)
                    gf = st["gath"][:].bitcast(F16)  # [128, NGmax*G, 32]
                    for g in range(NG[t]):
                        rhs = ident_t[:] if g < C0 else st["ohs"][g - C0][:]
                        # psAT[je*32+c, m] += sum_k payload[k, je*32+c]*route[k, m]
                        nc.tensor.matmul(psATp[:, i, :], gf[:, g * G:(g + 1) * G, :],
                                         rhs, start=(g == 0), stop=(g == NG[t] - 1))
                sbTp = outp.tile([128, 2, 128], F16, tag="sbT", name="sbTp")
                nc.scalar.activation(sbTp[:], psATp[:], mybir.ActivationFunctionType.Copy)
                psZp = psZ_p.tile([128, 2, H], F32, tag="psZ",
                                  padded_shape=[128, 2, 256], name="psZp")
                for i in range(2):
                    # z2[m, h] = sum_{je,c} aggT[je*32+c, m] * upwT4[je*32+c, h]
                    nc.tensor.matmul(psZp[:, i, :], sbTp[:, i, :], upwT_t[:],
                                     start=True, stop=not cfg.use_c)
                    if cfg.use_c:
                        nc.tensor.matmul(psZp[:, i, :], ones1[:], c_t[:], start=False,
                                         stop=True)
                live[("psZ", P)] = psZp

            def stage_epi1(P):
                psZp = live.pop(("psZ", P))
                vp = outp.tile([128, 2, H], F16, tag="v", name="vp")
                if P % 2 == 0:
                    nc.vector.tensor_scalar_max(vp[:], psZp[:], 0.0)
                else:
                    nc.scalar.activation(vp[:], psZp[:],
                                         mybir.ActivationFunctionType.Relu)
                v2p = outp.tile([128, 2, H], F16, tag="v2", name="v2p")
                nc.gpsimd.tensor_tensor(v2p[:], vp[:], eta_all[:, 2 * P:2 * P + 2, :],
                                        mybir.AluOpType.add)
                if P % 2 == 0:
                    mvq = statp.tile([128, 4, 2], F32, tag="mv", name="mvq")
                    live[("mvq", P // 2)] = mvq
                mvq = live[("mvq", P // 2)]
                for i in range(2):
                    st6 = statp.tile([128, 6], F32, tag="st", name="st6")
                    nc.vector.bn_stats(st6[:], v2p[:, i, :])
                    nc.vector.bn_aggr(mvq[:, 2 * (P % 2) + i, :], st6[:])
                if P % 2 == 1:
                    # one Sqrt/reciprocal for the whole quad (4 tiles)
                    sdq = statp.tile([128, 4, 1], F32, tag="sd", name="sdq")
                    nc.scalar.activation(sdq[:], mvq[:, :, 1:2],
                                         mybir.ActivationFunctionType.Sqrt,
                                         bias=eps_t[:])
                    live[("sdq", P // 2)] = sdq
                live[("epi", P)] = v2p

            def stage_epi2(P):
                v2p = live.pop(("epi", P))
                mvq = live[("mvq", P // 2)]
                if P % 2 == 0:
                    sdq = live[("sdq", P // 2)]
                    rsq = statp.tile([128, 4, 1], F32, tag="rs", name="rsq")
                    nc.vector.reciprocal(rsq[:], sdq[:])
                    live[("rsq", P // 2)] = rsq
                rsq = live[("rsq", P // 2)]
                for i in range(2):
                    t = 2 * P + i
                    j = 2 * (P % 2) + i
                    yt = y_all[:, t, :]
                    nc.vector.tensor_scalar(
                        yt, v2p[:, i, :], mvq[:, j, 0:1], rsq[:, j, 0:1],
                        mybir.AluOpType.subtract, mybir.AluOpType.mult,
                    )
                    if cfg.use_post_gb:
                        nc.vector.tensor_tensor(yt, yt, postg_b[:], mybir.AluOpType.mult)
                        nc.vector.tensor_tensor(yt, yt, postb_b[:], mybir.AluOpType.add)
                if P % 2 == 1:
                    q = P // 2
                    nc.sync.dma_start(y_dst[:, 4 * q:4 * q + 4, :],
                                      y_all[:, 4 * q:4 * q + 4, :])

            NP = T // 2
            for t in range(4):
                stage_gather(t)
            for P in range(NP):
                if 2 * P + 4 < T:
                    stage_gather(2 * P + 4)
                if 2 * P + 5 < T:
                    stage_gather(2 * P + 5)
                stage_route(P)
                if P >= 1:
                    stage_epi1(P - 1)
                if P >= 2:
                    stage_epi2(P - 2)
            stage_epi1(NP - 1)
            stage_epi2(NP - 2)
            stage_epi2(NP - 1)

    nc.compile()
    return nc


# ---------------------------------------------------------------------------
# host-side prep
# ---------------------------------------------------------------------------


def prep_inputs(x, edge_index, down_w, down_b, up_w, up_b, pre_g, pre_b, post_g,
                post_b, cfg=None):
    N = x.shape[1]
    H = x.shape[2]
    B = down_w.shape[0]
    src = np.asarray(edge_index[0], dtype=np.int64)
    dst = np.asarray(edge_index[1], dtype=np.int64)

    if cfg is None:
        cfg = Cfg(N=N, H=H, B=B)
    G, C0 = cfg.G, cfg.C0
    cap = C0 * G
    T = cfg.T
    n_tiles_total = N // 128

    order = np.argsort(src, kind="stable")
    src_s = src[order]
    dst_s = dst[order]
    tile_of = (src_s >> 7).astype(np.int64)
    row = (src_s & 127).astype(np.int64)
    cnt = np.bincount(src_s, minlength=N)
    row_start = np.concatenate([[0], np.cumsum(cnt)])
    pos_in_row = np.arange(len(src_s)) - row_start[src_s]

    is_id = pos_in_row < cap
    # identity slots: claim k == row, group g = pos//G
    slot_id = ((pos_in_row // G) * G + pos_in_row % G) * 128 + row  # == pos*? no:
    slot_id = (pos_in_row // G) * G * 128 + (pos_in_row % G) * 128 + row

    # overflow claims, packed per tile in row order
    ov_pos = pos_in_row - cap
    ov_cpr = np.ceil(np.maximum(cnt - cap, 0) / G).astype(np.int64)  # per src row
    ov_cpr_t = ov_cpr.reshape(n_tiles_total, 128)
    ov_claim_base = np.cumsum(ov_cpr_t, axis=1) - ov_cpr_t
    ov_claims_per_tile = ov_cpr_t.sum(axis=1)
    ov_claim_idx = ov_claim_base[tile_of, row] + np.maximum(ov_pos, 0) // G
    g_ov = C0 + ov_claim_idx // 128
    k_ov = ov_claim_idx % 128
    slot_ov = (g_ov * G + ov_pos % G) * 128 + k_ov

    slot_of = np.where(is_id, slot_id, slot_ov)

    ngs_ov = np.maximum(0, -(-ov_claims_per_tile // 128)).reshape(cfg.NCORES, T)
    cfg.ngs_ov = tuple(int(v) for v in ngs_ov.max(axis=0))
    NG = [C0 + o for o in cfg.ngs_ov]
    cfg.use_bM = bool(np.any(pre_b != 0))
    cfg.use_c = bool(np.any(down_b != 0) or np.any(up_b != 0))
    cfg.use_pre_gb = bool(np.any(pre_g != 1) or np.any(pre_b != 0))
    cfg.use_post_gb = bool(np.any(post_g != 1) or np.any(post_b != 0))

    bounds = np.searchsorted(tile_of, np.arange(n_tiles_total + 1))
    tot_ov = sum(cfg.ngs_ov)

    ident = np.eye(128, dtype=np.float16)
    iota = np.tile(np.arange(128, dtype=np.float16), (128, 1))
    dw_eff = (np.asarray(down_w, np.float32) * np.asarray(pre_g, np.float32)[None, :])
    dwT = np.ascontiguousarray(dw_eff.T).astype(np.float16)  # [H, B]
    c1 = (np.asarray(pre_b, np.float32) @ dw_eff.T).reshape(1, B).astype(np.float16)
    upwT4 = np.tile(np.asarray(up_w, np.float32).T, (128 // B, 1)).astype(np.float16)
    c_t = (np.asarray(down_b, np.float32) @ np.asarray(up_w, np.float32).T
           + np.asarray(up_b, np.float32)).reshape(1, H).astype(np.float16)

    k1_maps, k2_maps = [], []
    for c in range(cfg.NCORES):
        fl_parts = []
        Rp = np.zeros((128, max(tot_ov, 1)), np.float32)
        ovcol = 0
        for t in range(T):
            gt = c * T + t
            a, b = bounds[gt], bounds[gt + 1]
            n_slots = NG[t] * G * 128
            fl = np.full(n_slots, N, dtype=np.int64)
            fl[slot_of[a:b]] = dst_s[a:b]
            fl_parts.append(np.tile(fl.reshape(-1, 16).T, (8, 1)))
            ncl = ov_claims_per_tile[gt]
            if ncl:
                rr = np.repeat(np.arange(128), ov_cpr_t[gt])  # src row per ov claim
                ci = np.arange(ncl)
                Rp[ci % 128, ovcol + ci // 128] = rr
            ovcol += cfg.ngs_ov[t]
        idx_plane = np.ascontiguousarray(
            np.concatenate(fl_parts, axis=1).astype(np.int16))

        k1_maps.append({
            "x_slab": np.ascontiguousarray(
                x[0, c * cfg.SLAB:(c + 1) * cfg.SLAB, :]).astype(np.float16),
            "ident": ident,
            "dwT": dwT,
            "c1": c1,
            "pre_g": np.asarray(pre_g, np.float16).reshape(1, H),
            "pre_b": np.asarray(pre_b, np.float16).reshape(1, H),
        })
        k2_maps.append({
            "idx_plane": idx_plane,
            "R_plane": np.ascontiguousarray(Rp),
            "iota": iota,
            "ident": ident,
            "upwT4": upwT4,
            "c_t": c_t,
            "post_g": np.asarray(post_g, np.float16).reshape(1, H),
            "post_b": np.asarray(post_b, np.float16).reshape(1, H),
        })
    return cfg, k1_maps, k2_maps


def table_from_d(cfg, d_list):
    """d_list: per-core [128, T*B] f16 -> int32 table [N+1, 64] (256B rows)."""
    B = cfg.B
    parts = []
    for arr in d_list:
        a = np.asarray(arr).astype(np.float16).reshape(128, cfg.T, B)
        parts.append(np.ascontiguousarray(a.transpose(1, 0, 2)).reshape(cfg.SLAB, B))
    d_full = np.concatenate(parts, axis=0)
    tabf = np.zeros((cfg.N + 1, 128), np.float16)
    tabf[:cfg.N, :B] = d_full
    return tabf.view(np.int32)


def y_from_outs(cfg, y_list):
    """per-core [128, T*H] f16 -> [1, N, H] f32."""
    H = cfg.H
    parts = []
    for arr in y_list:
        a = np.asarray(arr).astype(np.float32).reshape(128, cfg.T, H)
        parts.append(np.ascontiguousarray(a.transpose(1, 0, 2)).reshape(cfg.SLAB, H))
    return np.concatenate(parts, axis=0)[None]


# ---------------------------------------------------------------------------
# main entry
# ---------------------------------------------------------------------------

_CACHE = {}


def _run_spmd(nc, maps, cores):
    try:
        import jax
        jax.config.update("jax_enable_x64", True)
    except Exception:
        pass
    from concourse.bass_utils import run_bass_kernel_spmd

    last_err = None
    for _attempt in range(3):
        try:
            return run_bass_kernel_spmd(nc, maps, cores).results
        except Exception as e:  # transient device/transport errors
            last_err = e
            import time as _time
            _time.sleep(2.0)
    raise last_err


def kernel(x, edge_index, down_w, down_b, up_w, up_b, pre_g, pre_b, post_g, post_b):
    import numpy as _np

    inputs = dict(x=_np.asarray(x), edge_index=_np.asarray(edge_index),
                  down_w=_np.asarray(down_w), down_b=_np.asarray(down_b),
                  up_w=_np.asarray(up_w), up_b=_np.asarray(up_b),
                  pre_g=_np.asarray(pre_g), pre_b=_np.asarray(pre_b),
                  post_g=_np.asarray(post_g), post_b=_np.asarray(post_b))
    cfg, k1_maps, k2_maps = prep_inputs(**inputs)
    key = (cfg.N, cfg.H, cfg.B, cfg.G, cfg.C0, cfg.ngs_ov, cfg.use_bM, cfg.use_c,
           cfg.use_pre_gb, cfg.use_post_gb)
    if key not in _CACHE:
        _CACHE[key] = (build_k1(cfg), build_k2(cfg))
    nc1, nc2 = _CACHE[key]
    cores = list(range(cfg.NCORES))
    r1 = _run_spmd(nc1, k1_maps, cores)
    table = table_from_d(cfg, [r1[c]["d"] for c in range(cfg.NCORES)])
    for c in range(cfg.NCORES):
        k2_maps[c]["table"] = table
        k2_maps[c]["eta"] = r1[c]["eta"]
    r2 = _run_spmd(nc2, k2_maps, cores)
    return y_from_outs(cfg, [r2[c]["y"] for c in range(cfg.NCORES)]).astype(_np.float32)
